# revision 1
# baseline (speedup 1.0000x reference)
"""CARAFE content-aware upsampling on 8 Trainium2 NeuronCores (Bass/Tile).

Problem: features (4,128,64,64) f32, masks (4,25,128,128) f32
         -> out (4,128,128,128) f32
out[n,c,2h+a,2w+b] = sum_{i,j in 5x5} f[n,c,h+i-2,w+j-2] * m[n,5i+j,2h+a,2w+b]

Strategy (per core = one (n, h-half) shard):
  For each low-res row h we compute out[c, (a, wup)] (two upsampled rows,
  256 cols) as 5 PSUM-accumulated fp32r matmuls, one per kernel-row i:
     out += fT_row(h+i-2).T @ B_i
  where fT_row is the W-padded transposed feature row [w''(68), c(128)]
  (host-pretransposed) and B_i [w''(68), 256 cols] is a banded matrix
  holding the masks on diagonals.  Band columns are laid out (w, b, a) so
  each partition's band content is one contiguous 20-element (80 B) run;
  the matmul rhs reads it back as (a, wup) via a stride-2 inner AP.
  Bands are materialized by a per-job SBUF->SBUF diagonal-scatter DMA
  (dest AP steps +1 partition +4 elements) out of a bulk-loaded staging
  copy of the host-rearranged masks.  The band sparsity pattern is
  static, so the zero background is memset once and runs are overwritten
  in place; run overrun at the edges lands in 16-element pad gaps
  between the five band regions.
"""
import sys

if "/opt/trn_rl_repo" not in sys.path:
    sys.path.insert(0, "/opt/trn_rl_repo")

from contextlib import ExitStack

import numpy as np

import concourse.tile as tile
from concourse import bacc, mybir
from concourse.ap import AP
from concourse.bass_utils import run_bass_kernel_spmd

# ---- problem constants (hardcoded per harness contract) ----
N = 4
C = 128
H = 64
W = 64
KS = 5
PAD = 2
SCALE = 2
WP = W + KS - 1          # 68 contraction width per feature row
NB = SCALE * W           # 128 upsampled cols per hup row
RUN = 4 * KS             # 20 elems per diagonal run (w,b,a interleaved)
REG = 2 * NB + 32        # 288 per-band region: 16 pad | 256 data | 16 pad
BW = KS * REG            # 1440 band buffer free width
NH = H // 2              # 32 low-res rows per core
NROWS = NH + 4           # 36 feature rows per shard (halo zero-padded)
N_BBUF = 8
OBATCH = 8               # jobs per output DMA

F32 = mybir.dt.float32
F32R = mybir.dt.float32r

_PROG_CACHE: dict = {}


def _device_body(tc, ctx, out_ap, ft_ap, msk3_ap):
    nc = tc.nc
    if True:
        sb = ctx.enter_context(tc.tile_pool(name="sb", bufs=1))
        psum = ctx.enter_context(tc.tile_pool(name="ps", bufs=4, space="PSUM"))
        obp = ctx.enter_context(tc.tile_pool(name="ob", bufs=3))

        # chunked input loads, spread across both HWDGE rings so job 0's
        # data lands early and loads overlap compute
        ft = sb.tile([WP, NROWS * C], F32)
        mst = sb.tile([WP, NH * KS * RUN], F32)
        mstap = mst[:]
        MCH = 4 * KS * RUN                     # mask cols per 4-job chunk
        n_mch = NH // 4
        ft_bounds = [0, 11, 20, 29, NROWS]     # rows: jobs 0-6 / -15 / -24 / -31
        mch, fch = 0, 0
        order = [("m", 0), ("f", 0), ("m", 1), ("f", 1), ("m", 2), ("f", 2),
                 ("m", 3), ("f", 3)] + [("m", g) for g in range(4, n_mch)]
        for k2, (kind, g) in enumerate(order):
            eng = (nc.sync, nc.scalar, nc.gpsimd)[min(k2, 2)]
            if kind == "m":
                eng.dma_start(
                    mst[:, g * MCH : (g + 1) * MCH],
                    msk3_ap[:, g * MCH : (g + 1) * MCH],
                )
                mch += 1
            else:
                lo, hi = ft_bounds[g] * C, ft_bounds[g + 1] * C
                eng.dma_start(
                    ft[:, lo:hi].bitcast(F32R), ft_ap[:, lo:hi].bitcast(F32R)
                )
                fch += 1

        # persistent band buffers, memset once (static sparsity pattern)
        bbufs = []
        for q in range(N_BBUF):
            b = sb.tile([WP, BW], F32, tag=f"bbuf{q}")
            nc.vector.memset(b[:], 0.0)
            bbufs.append(b)

        ob4 = None
        for hl in range(NH):
            bap = bbufs[hl % N_BBUF][:]
            # SBUF->SBUF diagonal scatter: all 5 bands' runs for this job.
            # dest: [w' (+1 part,+4 col)][i: region][t: run]
            dst = AP(bap.tensor, bap.offset, [[BW + 4, WP], [REG, KS], [1, RUN]])
            src = AP(
                mstap.tensor,
                mstap.offset + hl * KS * RUN,
                [[NH * KS * RUN, WP], [RUN, KS], [1, RUN]],
            )
            if hl % 2 == 0:
                eng = nc.sync if (hl // 2) % 2 == 0 else nc.scalar
            else:
                eng = nc.gpsimd
            eng.dma_start(dst.bitcast(F32R), src.bitcast(F32R))

            ps = psum.tile([C, 2 * NB], F32)
            for i in range(KS):
                lhsT = ft[:, (hl + i) * C : (hl + i + 1) * C].bitcast(F32R)
                rhs = AP(
                    bap.tensor,
                    bap.offset + i * REG + 16,
                    [[BW, WP], [1, 2], [2, NB]],
                ).bitcast(F32R)
                nc.tensor.matmul(ps[:], lhsT, rhs, start=(i == 0), stop=(i == 4))

            if hl % OBATCH == 0:
                ob4 = obp.tile([C, OBATCH * 2 * NB], F32)
            sl = ob4[:, (hl % OBATCH) * 2 * NB : (hl % OBATCH + 1) * 2 * NB]
            if hl % 2 == 0:
                nc.scalar.copy(sl, ps[:])
            else:
                nc.vector.tensor_copy(sl, ps[:])
            if hl == NH - 5:
                g = hl - (OBATCH - 5)
                nc.gpsimd.dma_start(
                    out_ap[:, 2 * g : 2 * g + 8, :], ob4[:, : 4 * 2 * NB]
                )
            elif hl == NH - 1:
                nc.scalar.dma_start(
                    out_ap[:, 2 * (NH - 4) : 2 * NH, :], ob4[:, 4 * 2 * NB :]
                )
            elif hl % OBATCH == OBATCH - 1:
                g = hl - (OBATCH - 1)
                nc.gpsimd.dma_start(
                    out_ap[:, 2 * g : 2 * g + 2 * OBATCH, :], ob4[:]
                )


def _build_program():
    nc = bacc.Bacc(
        "TRN2", debug=False, enable_asserts=False, target_bir_lowering=False
    )
    ft_t = nc.dram_tensor("ft", [WP, NROWS * C], F32, kind="ExternalInput")
    msk_t = nc.dram_tensor("msk3", [WP, NH * KS * RUN], F32, kind="ExternalInput")
    out_t = nc.dram_tensor("out", [C, 2 * NH, NB], F32, kind="ExternalOutput")

    with tile.TileContext(nc) as tc, ExitStack() as ctx:
        _device_body(tc, ctx, out_t.ap(), ft_t.ap(), msk_t.ap())
    nc.compile()
    return nc


def _prep_ft(feat_n: np.ndarray, h0: int) -> np.ndarray:
    """[C,H,W] -> fT[w'', r, c] with r over [h0-2, h0+NH+2), zero-padded."""
    ft = np.zeros((WP, NROWS, C), np.float32)
    r_lo, r_hi = h0 - 2, h0 + NH + 2
    s_lo, s_hi = max(r_lo, 0), min(r_hi, H)
    # f[c, r, w] -> [w, r, c]
    ft[PAD : PAD + W, s_lo - r_lo : s_hi - r_lo, :] = feat_n[:, s_lo:s_hi, :].transpose(
        2, 1, 0
    )
    return np.ascontiguousarray(ft.reshape(WP, NROWS * C))


def _prep_msk3(masks_n: np.ndarray) -> np.ndarray:
    """[25, 2H, 2W] -> msk3[w', h, i, t20]  [WP, H, KS, RUN]
    t20 = (w - (w'-4))*4 + b*2 + a; value = masks[5i + (4 - t20//4), 2h+a, 2w+b]
    """
    tt = np.arange(RUN)
    wpp = np.arange(WP)
    dw = tt // 4
    b = (tt % 4) // 2
    a = tt % 2
    j = 4 - dw
    wup = 2 * (wpp[:, None] - 4 + dw[None, :]) + b[None, :]
    wup_c = np.clip(wup, 0, 2 * W - 1)                     # [WP, RUN]
    i_ar = np.arange(KS)
    k_full = 5 * i_ar[:, None] + j[None, :]                # [KS, RUN]
    hh = np.arange(H)
    hup = 2 * hh[:, None] + a[None, :]                     # [H, RUN]
    out = masks_n[
        k_full[None, None, :, :],
        hup[None, :, None, :],
        wup_c[:, None, None, :],
    ]  # [WP, H, KS, RUN]
    return np.ascontiguousarray(out.astype(np.float32))


def kernel(features: np.ndarray, masks: np.ndarray, _perf: dict | None = None):
    features = np.asarray(features, dtype=np.float32)
    masks = np.asarray(masks, dtype=np.float32)

    if "nc" not in _PROG_CACHE:
        _PROG_CACHE["nc"] = _build_program()
    nc = _PROG_CACHE["nc"]

    in_maps = []
    for core in range(8):
        n, half = divmod(core, 2)
        h0 = NH * half
        ft_sh = _prep_ft(features[n], h0)
        msk3 = _prep_msk3(masks[n])[:, h0 : h0 + NH]  # [WP, NH, KS, RUN]
        in_maps.append(
            {
                "ft": ft_sh,
                "msk3": np.ascontiguousarray(msk3.reshape(WP, NH * KS * RUN)),
            }
        )

    trace = bool(_perf is not None and _perf.get("trace"))
    res = run_bass_kernel_spmd(
        nc, in_maps, core_ids=list(range(8)), trace=trace,
        **({} if not trace else {"trace_cores": [0]}),
    )
    if _perf is not None:
        _perf["exec_time_ns"] = res.exec_time_ns
        _perf["trace"] = res.instructions_and_trace

    out = np.empty((N, C, SCALE * H, SCALE * W), np.float32)
    for core in range(8):
        n, half = divmod(core, 2)
        out[n, :, 64 * half : 64 * half + 64, :] = res.results[core]["out"]
    return out

